# revision 66
# baseline (speedup 1.0000x reference)
"""Trainium2 Bass kernel for a fused GRUCell step.

Math (reference):
    xi = x @ [W_ir W_iz W_in] + [b_ir b_iz b_in]
    hh = h @ [W_hr W_hz W_hn]
    r = sigmoid(xr + hr); z = sigmoid(xz + hz)
    n = tanh(xn + r * (hn + b_hn))
    new_h = (1 - z) * n + z * h

Strategy (v14, HW-measured ~229.4us mean of 4, rel err 1.98387e-2;
v11 242us,
v5 274.6us, v1 397us). HW exec noise between identical runs is
+-1.5-3us — judge changes on multi-run means, and only via the test.py
flow (one untraced run + one traced run per process; back-to-back
traced runs in one process measure ~45us slower for unknown
environmental reasons). Rel err is DETERMINISTIC (identical to 6
digits across runs) and the numpy sim (sim_precision.py method) now
predicts HW rel err to 6 digits at full batch — precision headroom can
be spent exactly to the 2e-2 gate.

- 2D shard — batch 4-way x hidden 2-way (8 cores), each core a
  [B_CORE=4096, H_CORE=512] output tile. Weights per core ~3.25MB.
  The GEMM runs with the *weights* stationary and 512-wide batch blocks
  moving. Outputs land transposed ([H-partition, batch-free]); h is
  supplied host-transposed and the final unshard transposes back.
  Per-gate biases become per-partition scalars, fused into ScalarE
  activations for free.
- Precision (v6): only the xz and xn sweeps stay fp16 (1 cycle/row on
  the PE); r (K=2048), hn (K=1024) and now hz (K=1024) run fp8(e4m3)
  with perf_mode=DoubleRow, packing K=256 per matmul — 2x PE throughput
  (HW-verified: DR matmuls issue at the same 216ns slot as fp16, so DR
  halves the slot count). 36 -> 32 slots per chunk. Weights are
  pre-scaled by WS=16 into the e4m3 normal range; the xz fp16 weights
  are also scaled by WS (exact, power of two) so the z PSUM
  accumulation is uniformly scaled and one sigmoid activation scale
  1/WS folds the rescale for free. Numpy-simulated rel err 1.763e-2
  against the 2e-2 gate (sim matched HW to ~4 digits at v5); full-fp8 z
  (2.18e-2) does not fit.
- Chunks are processed in PAIRS sharing one DR block and one fp16
  block (v7): hn0 hz0 r0 hn1 hz1 r1 | xn0 xz0 xn1 xz1. The fp16->fp8
  PE mode flip costs ~500ns (HW-measured, lands on the 2nd matmul
  after the flip); pairing halves the flip count 32 -> 16. All eight
  PSUM banks are used single-buffered (pr/pz/pxn/phn x 2 chunks); the
  tile framework's slice-level tracking makes the next pair's sweeps
  wait only for the previous pair's epilogue reads of the same bank,
  which complete mid-pair (~0.7us margin, HW-verified no new stalls).
  z's PSUM accumulates across the hz (DR) and xz (fp16) sweeps into
  one bank and finishes last, so the whole tanh chain overlaps the xz
  sweep. hn leads the pair so the first real matmul of the kernel
  needs only 384KB of operands (w8 hn+hz part + lt8 h-half).
- Startup DMAs ride two HWDGE queues in parallel (v8): weights on the
  sync queue, lhsT blocks on the scalar (Activation) queue — the
  descriptor-issue instructions are ~650ns serialized per queue plus a
  ~2.6us ring latency, so splitting pulls the first real matmul's
  operand arrival ~3us earlier. (A second queue does NOT help steady
  bandwidth — one queue saturates the HBM port — and the scalar queue
  is idle during startup.)
- Weight/lhsT tiles beyond chunk 0 load as SINGLE whole-tile DMAs
  (4.6-8KB segments run the DMA engines ~1.5x faster than 2KB-segment
  halves); only chunk 0 keeps the split for first-matmul latency.
- DMA issue order is hand-scheduled on the sync queue (one FIFO,
  ~380GB/s measured once ramped, bulk flow from ~8us): w8(0) is split
  in two so the hn/hz part lands first, then the lt8 h-half, the r
  part, lt8 x-half, wxn, lt halves, wxz. Remaining weights stream in
  consumption order during block 0, lhsT blocks prefetch mid-block.
- ~48 warmup matmuls on a zeroed scratch tile run during the startup
  DMA wait so the PE HAM clock gate is at 2.4GHz (not 1.2) when the
  real stream starts. v5 used 100 and the warmup tail (15.3us) delayed
  the real stream past operand arrival (~10.8us). All four PSUM pool
  tags must be rotated together: rotating only one puts it anti-phase
  with the others and serializes every chunk against the epilogue
  (+50us when tried).
- Blocks walk the hidden chunks in snake order (odd blocks reversed),
  so block 1's first chunk needs the weights that arrive last — this
  removed a 4.4us stall at the block 0->1 boundary.
- The last chunk's epilogue runs in two 256-column pieces to shorten
  the post-matmul tail.
- Epilogue (v11): sr/sz/sn/tt intermediates are fp16 (2x DVE
  throughput, |r|,|z|,|n|<=1 so rounding is ~3e-4). Six ops + three
  activations per chunk.
- v12/v13: the ENTIRE n-path (hn fp8 + xn fp16 weights) carries the
  WS scale and one tanh activation scale folds the 1/WS for free, so
  hn quantizes in the e4m3 normal range with no rescale op. The last
  512 K-rows of the xz GEMM moved into the DR block as 2 extra fp8
  chunks per hidden chunk (the fp8 moving slices are already in SBUF
  for the r sweep): 32 -> 30 slots per chunk, -13.8us. This spends
  the precision budget to 1.98387e-2 vs the 2e-2 gate — deterministic,
  sim-verified to 6 digits at full batch; a third xz quarter (2.08e-2)
  and an xn quarter (2.04e-2) do NOT fit.

Accumulation is fp32 in PSUM throughout: pr/pz/pxn/phn x 2 chunks of
the active pair = all 8 banks, single-buffered.

Measured anatomy at v11 (exec ~242us): entry barrier + engine preamble
to ~7.2us, first real matmul ~12.8us (DMA-bandwidth-bound startup:
pair 0 consumes 4.25MB at the ~0.31MB/us early DMA rate), matmul
stream 1024 x 216ns + ~6us of flip/boundary stalls, ends ~235us;
epilogue+out-DMA tail ~3.5us; fixed teardown barrier ~4us.
"""

import os
import sys

import numpy as np

sys.path.insert(0, "/opt/trn_rl_repo")
os.environ.setdefault("MYCRO_LOCAL_CACHE", "1")

import concourse.bass as bass  # noqa: E402
import concourse.mybir as mybir  # noqa: E402
import concourse.tile as tile  # noqa: E402
from concourse import bacc  # noqa: E402
from concourse.bass_utils import run_bass_kernel_spmd  # noqa: E402

N_CORES = 8
B_SHARDS = 4
H_SHARDS = 2
B = 16384
F = 1024  # input feature dim
H = 1024  # hidden dim
K = F + H  # full GEMM contraction dim (x features then h features)
P = 128
KOX = F // P  # 8 k-chunks of 128 for the fp16 (x-side) sweeps
B_CORE = B // B_SHARDS  # 4096
H_CORE = H // H_SHARDS  # 512
MBLK = 512  # batch columns per moving-operand block (= PSUM bank width)
NBLK = B_CORE // MBLK  # 8
HC_N = H_CORE // P  # 4 hidden-column chunks of 128 (PSUM partition dim)
KO8 = K // 256  # 8 k-chunks of 256 for fp8 DoubleRow (r gate, full K)
KO8X = F // 256  # 4 fp8 k-chunks belonging to the x part
KO8H = H // 256  # 4 fp8 k-chunks for the hn / hz parts
XZ8_N = 2  # fp8 DR chunks of the xz part (x rows 512..1024, v13)
W8_N = KO8 + 2 * KO8H + XZ8_N  # 18 fp8 weight chunks: r | hn | hz | xz-half
HN0 = KO8  # first hn chunk index in w8rh
HZ0 = KO8 + KO8H  # first hz chunk index in w8rh
XZ8 = HZ0 + KO8H  # first xz fp8 chunk index in w8rh
KOXZ = KOX - 2 * XZ8_N  # 4 fp16 xz k-chunks (rows 0..512; the rest fp8 DR)
WS = 16.0  # fp8 weight scale: lifts w std ~0.031 into e4m3 normal range


def build_gru_program(with_bias: bool) -> bass.Bass:
    """One SPMD program; every core runs it on its own (batch, hidden) tile."""
    fp16 = mybir.dt.float16
    f32 = mybir.dt.float32

    # Bacc (not plain Bass): its compile pipeline splits multi-sem waits into
    # event semaphores — walrus rejects >1 wait on most engine instructions.
    nc = bacc.Bacc()
    # Host-prearranged so every DMA lands contiguously per partition:
    # lhsT[b, p, ko, m] = x.T[ko*P+p, b*MBLK+m]  (x rows only; fp16 sweeps)
    lhsT = nc.declare_dram_parameter("lhsT", [NBLK, P, KOX, MBLK], fp16, isOutput=False)
    fp8 = mybir.dt.float8e4
    # xz and xn (first F rows of z / n gates) stay fp16; xz pre-scaled by WS
    wxz = nc.declare_dram_parameter("wxz", [HC_N, P, KOXZ, P], fp16, isOutput=False)
    wxn = nc.declare_dram_parameter("wxn", [HC_N, P, KOX, P], fp16, isOutput=False)
    # r gate (full K) + hn + hz parts in fp8 DoubleRow layout, combined:
    # j in [0,KO8):      WS*Wcat_r[j*256+i*128+p, hc*P+n]
    # j in [HN0,HN0+4):  W_hn[(j-HN0)*256+i*128+p, hc*P+n]   (unscaled)
    # j in [HZ0,HZ0+4):  WS*W_hz[(j-HZ0)*256+i*128+p, hc*P+n]
    w8rh = nc.declare_dram_parameter("w8rh", [HC_N, P, W8_N, 2, P], fp8, isOutput=False)
    # fp8 copy of the moving operand (full K), DoubleRow-paired
    lhsT8 = nc.declare_dram_parameter(
        "lhsT8", [NBLK, P, KO8, 2, MBLK], fp8, isOutput=False
    )
    # hT[j, m] = h_shard[m, j]  (host-transposed h slice; fp16 — only feeds
    # the z*h blend, where fp16 rounding of h is ~3e-4 relative)
    hT = nc.declare_dram_parameter("hT", [H_CORE, B_CORE], fp16, isOutput=False)
    if with_bias:
        # bias[p, g, hc] = b_g[hj*H_CORE + hc*P + p]; g: 0=b_ir 1=b_iz 2=b_in 3=b_hn
        biasp = nc.declare_dram_parameter("bias", [P, 4, HC_N], f32, isOutput=False)
    out = nc.declare_dram_parameter("out", [H_CORE, B_CORE], f32, isOutput=True)

    Sigmoid = mybir.ActivationFunctionType.Sigmoid
    Tanh = mybir.ActivationFunctionType.Tanh
    DR = mybir.MatmulPerfMode.DoubleRow

    with tile.TileContext(nc) as tc:
        with (
            tc.tile_pool(name="wpool", bufs=1) as wpool,
            tc.tile_pool(name="lpool", bufs=3) as lpool,
            tc.tile_pool(name="hpool", bufs=2) as hpool,
            tc.tile_pool(name="opool", bufs=3) as opool,
            tc.tile_pool(name="epool", bufs=2) as epool,
            tc.tile_pool(name="psum", bufs=1, space="PSUM") as psum,
        ):
            wsb = {}
            w8sb = {}

            def load_wxz(hc: int):
                t = wpool.tile([P, KOXZ, P], fp16, tag=f"wxz_{hc}")
                nc.sync.dma_start(t[:], wxz[hc])
                wsb[("xz", hc)] = t

            def load_wxn(hc: int):
                t = wpool.tile([P, KOX, P], fp16, tag=f"wxn_{hc}")
                nc.sync.dma_start(t[:], wxn[hc])
                wsb[("xn", hc)] = t

            def load_w8(hc: int, part: int, engine: str = "sync"):
                # part 1 = hn+hz+xz8 chunks (consumed first), part 0 = r
                # chunks, part 2 = whole tile in ONE dma (4.6KB segments run
                # the DMA engines ~1.5x faster than the 2KB-segment halves —
                # use for every chunk except the startup-critical chunk 0)
                if hc not in w8sb:
                    t = wpool.tile([P, W8_N, 2, P], fp8, tag=f"w8_{hc}")
                    w8sb[hc] = t
                t = w8sb[hc]
                eng = getattr(nc, engine)
                if part == 2:
                    eng.dma_start(t[:], w8rh[hc])
                elif part == 1:
                    eng.dma_start(t[:, HN0:W8_N, :, :], w8rh[hc, :, HN0:W8_N, :, :])
                else:
                    eng.dma_start(t[:, 0:HN0, :, :], w8rh[hc, :, 0:HN0, :, :])

            def load_lt(b: int):
                # two ko-halves so the xn sweep can start after 0.5MB
                t = lpool.tile([P, KOX, MBLK], fp16, tag="lt")
                half = KOX // 2
                nc.sync.dma_start(t[:, 0:half, :], lhsT[b, :, 0:half, :])
                nc.sync.dma_start(t[:, half:KOX, :], lhsT[b, :, half:KOX, :])
                return t

            # --- startup-critical DMA order ---
            # Two HWDGE queues issue in parallel (descriptor-issue is ~650ns
            # serialized per queue + ~2.6us ring latency): the sync queue
            # carries the weights, the scalar (Activation) queue carries the
            # lhsT blocks. Pair 0 runs hn(0) first: needs w8(0) part1 +
            # lt8 h-half = 384KB, one descriptor each on the two queues.
            # chunk 1's weights follow since the pair consumes them ~3.5us in.
            lt8 = lpool.tile([P, KO8, 2, MBLK], fp8, tag="lt8")
            # (GpSimd SWDGE for these two loads was tried — its engine exits
            # the preamble earlier but software descriptor generation is far
            # slower: 250.7us vs 243.9us. Stay on the HWDGE queues.)
            # (Splitting these first loads into smaller first-consumed slices
            # moved the first real matmul 11.5us <- 12.8us but the startup
            # window is DMA-BANDWIDTH-bound — pair 0 consumes 4.25MB at the
            # ~0.31MB/us early rate — so the extra descriptors only delayed
            # the later transfers: 245.3us vs 242.3us mean. Keep it coarse.)
            load_w8(0, 1)
            nc.scalar.dma_start(lt8[:, KO8X:KO8, :, :], lhsT8[0, :, KO8X:KO8, :, :])
            load_w8(0, 0)
            nc.scalar.dma_start(lt8[:, 0:KO8X, :, :], lhsT8[0, :, 0:KO8X, :, :])
            load_w8(1, 2)
            lt = lpool.tile([P, KOX, MBLK], fp16, tag="lt")
            nc.scalar.dma_start(lt[:, 0 : KOX // 2, :], lhsT[0, :, 0 : KOX // 2, :])
            nc.scalar.dma_start(lt[:, KOX // 2 : KOX, :], lhsT[0, :, KOX // 2 : KOX, :])
            load_wxn(0)
            load_wxz(0)
            load_wxn(1)
            load_wxz(1)
            bias_sb = None
            if with_bias:
                bias_sb = wpool.tile([P, 4, HC_N], f32, tag="bias_sb")
                nc.sync.dma_start(bias_sb[:], biasp[:])

            ht_tiles = {}

            def load_ht(b: int, hc: int):
                t = hpool.tile([P, MBLK], fp16, tag=f"ht{hc}")
                nc.sync.dma_start(
                    t[:], hT[hc * P : (hc + 1) * P, b * MBLK : (b + 1) * MBLK]
                )
                ht_tiles[(b, hc)] = t

            load_ht(0, 0)
            load_ht(0, 1)

            # PE warmup: ~48 tiny matmuls on a zeroed scratch tile while the
            # first operands stream in. Keeps the PE HAM activity window busy
            # so it unthrottles from 1.2GHz to 2.4GHz before the real matmul
            # stream starts, and ends (~11.6us) just as the operands land.
            # fp8 DR warmup (v15): matches the mode of the stream-opening DR
            # sweep. (The ~430ns slot-1 stall survives regardless — it is a
            # stream-start pipeline effect, NOT the fp16->fp8 flip; timing
            # measured identical to the fp16 warmup, kept for mode hygiene.)
            warm_w = wpool.tile([P, 2, P], fp8, tag="warm_w")
            nc.vector.memset(warm_w[:], 0.0)
            warm_ps = psum.tile([P, MBLK], f32, tag="pr0")
            for _ in range(48):
                nc.tensor.matmul(
                    warm_ps[:, 0:P],
                    warm_w[:],
                    warm_w[:, :, 0:P],
                    start=True,
                    stop=True,
                    perf_mode=DR,
                )

            def epilogue(b, hc, pr, pz, pxn, phn, ht, lo, hi):
                s = slice(lo, hi)
                # fp16 intermediates: 2x DVE throughput on the gate chain;
                # |r|,|z|,|n| <= 1 so fp16 rounding adds ~3e-4 rel (sim:
                # 1.7634e-2 vs 1.7630e-2). hn weights are unscaled (WS=1,
                # sim 1.790e-2) so the old 1/WS rescale op disappears.
                sr = epool.tile([P, MBLK], fp16, tag="sr")
                sz = epool.tile([P, MBLK], fp16, tag="sz")
                sn = epool.tile([P, MBLK], fp16, tag="sn")
                tt = epool.tile([P, MBLK], fp16, tag="tt")
                ot = opool.tile([P, MBLK], f32, tag="ot")
                # issue order matters: ScalarE/DVE are strict FIFO, so sz
                # (which waits on the last-finishing xz-sweep) is issued
                # after tanh to avoid blocking the chain
                if with_bias:
                    nc.scalar.activation(
                        sr[:, s],
                        pr[:, s],
                        Sigmoid,
                        bias=bias_sb[:, 0, hc : hc + 1],
                        scale=1.0 / WS,
                    )
                    nc.vector.tensor_scalar(
                        tt[:, s],
                        phn[:, s],
                        1.0,
                        bias_sb[:, 3, hc : hc + 1],
                        mybir.AluOpType.mult,
                        mybir.AluOpType.add,
                    )
                    nc.vector.tensor_mul(tt[:, s], sr[:, s], tt[:, s])
                    nc.vector.tensor_add(tt[:, s], tt[:, s], pxn[:, s])
                    nc.scalar.activation(
                        sn[:, s],
                        tt[:, s],
                        Tanh,
                        bias=bias_sb[:, 2, hc : hc + 1],
                        scale=1.0 / WS,
                    )
                    nc.vector.tensor_sub(tt[:, s], ht[:, s], sn[:, s])
                    nc.scalar.activation(
                        sz[:, s],
                        pz[:, s],
                        Sigmoid,
                        bias=bias_sb[:, 1, hc : hc + 1],
                        scale=1.0 / WS,
                    )
                else:
                    nc.scalar.activation(sr[:, s], pr[:, s], Sigmoid, scale=1.0 / WS)
                    nc.vector.tensor_mul(tt[:, s], sr[:, s], phn[:, s])
                    nc.vector.tensor_add(tt[:, s], tt[:, s], pxn[:, s])
                    # the whole n-path (hn, xn) carries the WS weight scale;
                    # one tanh activation scale rescales it for free (v12)
                    nc.scalar.activation(sn[:, s], tt[:, s], Tanh, scale=1.0 / WS)
                    nc.vector.tensor_sub(tt[:, s], ht[:, s], sn[:, s])
                    nc.scalar.activation(sz[:, s], pz[:, s], Sigmoid, scale=1.0 / WS)
                nc.vector.tensor_mul(tt[:, s], tt[:, s], sz[:, s])
                nc.vector.tensor_add(ot[:, s], sn[:, s], tt[:, s])
                nc.sync.dma_start(
                    out[hc * P : (hc + 1) * P, b * MBLK + lo : b * MBLK + hi],
                    ot[:, s],
                )

            def get_ht(b, hc):
                if (b, hc) in ht_tiles:
                    return ht_tiles.pop((b, hc))
                t = hpool.tile([P, MBLK], fp16, tag=f"ht{hc}")
                nc.sync.dma_start(
                    t[:], hT[hc * P : (hc + 1) * P, b * MBLK : (b + 1) * MBLK]
                )
                return t

            lt_next = None
            lt8_next = None
            for b in range(NBLK):
                # snake order: odd blocks walk the pairs in reverse, so the
                # first pair of block 1 needs the weights that arrive last
                prs = [(0, 1), (2, 3)] if b % 2 == 0 else [(3, 2), (1, 0)]
                for pi, (c0, c1) in enumerate(prs):
                    if b == 0 and pi == 0:
                        # pull in the remaining weights + early h tiles while
                        # pair 0 runs
                        for hc in (2, 3):
                            load_w8(hc, 2)
                            load_wxn(hc)
                            load_wxz(hc)
                        load_ht(0, 2)
                        load_ht(0, 3)
                        load_ht(1, 3)  # block 1 starts at hc=3 (snake)
                        load_ht(1, 2)
                    # prefetch next batch block during the second pair
                    if pi == 1 and b + 1 < NBLK:
                        # single whole-tile DMAs: 8KB segments, ~1.5x faster
                        # per DMA engine than the 2-4KB split halves (the
                        # halves only matter for the startup-critical block 0)
                        lt8_next = lpool.tile([P, KO8, 2, MBLK], fp8, tag="lt8")
                        nc.sync.dma_start(lt8_next[:], lhsT8[b + 1])
                        lt_next = lpool.tile([P, KOX, MBLK], fp16, tag="lt")
                        nc.sync.dma_start(lt_next[:], lhsT[b + 1])

                    ht0 = get_ht(b, c0)
                    ht1 = get_ht(b, c1)

                    pr0 = psum.tile([P, MBLK], f32, tag="pr0")
                    pz0 = psum.tile([P, MBLK], f32, tag="pz0")
                    pxn0 = psum.tile([P, MBLK], f32, tag="pxn0")
                    phn0 = psum.tile([P, MBLK], f32, tag="phn0")
                    pr1 = psum.tile([P, MBLK], f32, tag="pr1")
                    pz1 = psum.tile([P, MBLK], f32, tag="pz1")
                    pxn1 = psum.tile([P, MBLK], f32, tag="pxn1")
                    phn1 = psum.tile([P, MBLK], f32, tag="phn1")

                    # (v9/v10 tried alternating DR-first / fp16-first pairs
                    # to halve the ~620ns fp16->fp8 PE flips; the fp16-first
                    # pairs defer their epilogues past the pair end and the
                    # resulting back-pressure cost more than the flips saved:
                    # 245.2us vs 242.3us. Keep every pair DR-first.)
                    last_pair = b == NBLK - 1 and pi == len(prs) - 1

                    def hn_sweep(hc, phn):
                        for j in range(KO8H):
                            nc.tensor.matmul(
                                phn[:],
                                w8sb[hc][:, HN0 + j, :, :],
                                lt8[:, KO8X + j, :, :],
                                start=(j == 0),
                                stop=(j == KO8H - 1),
                                perf_mode=DR,
                            )

                    def hz_sweep(hc, pz, first):
                        for j in range(KO8H):
                            nc.tensor.matmul(
                                pz[:],
                                w8sb[hc][:, HZ0 + j, :, :],
                                lt8[:, KO8X + j, :, :],
                                start=(first and j == 0),
                                stop=False,
                                perf_mode=DR,
                            )
                        # v12/v13: the last 512 K-rows of the xz part also
                        # run fp8 DR here (weights xWS like everything in
                        # pz), trimming the fp16 xz sweep 8 -> 4 matmuls.
                        # The moving slices (x rows 512..1024) are already
                        # in SBUF for the r sweep.
                        for j2 in range(XZ8_N):
                            nc.tensor.matmul(
                                pz[:],
                                w8sb[hc][:, XZ8 + j2, :, :],
                                lt8[:, KO8X - XZ8_N + j2, :, :],
                                start=False,
                                stop=False,
                                perf_mode=DR,
                            )

                    def r_sweep(hc, pr):
                        for j in range(KO8):
                            nc.tensor.matmul(
                                pr[:],
                                w8sb[hc][:, j, :, :],
                                lt8[:, j, :, :],
                                start=(j == 0),
                                stop=(j == KO8 - 1),
                                perf_mode=DR,
                            )

                    def xn_sweep(hc, pxn):
                        for j in range(KOX):
                            nc.tensor.matmul(
                                pxn[:],
                                wsb[("xn", hc)][:, j, :],
                                lt[:, j, :],
                                start=(j == 0),
                                stop=(j == KOX - 1),
                            )

                    def xz_sweep(hc, pz, first, split_last=False):
                        for j in range(KOXZ):
                            if split_last and j == KOXZ - 1:
                                # the pair's LAST fp16 matmul runs 448+64
                                # cols: the short tail MM drains the PE pipe
                                # ~190ns sooner, shrinking the fp16->fp8
                                # reconfigure stall of the next pair's DR
                                # block (if the stall is drain-limited)
                                for cs in (slice(0, 480), slice(480, 512)):
                                    nc.tensor.matmul(
                                        pz[:, cs],
                                        wsb[("xz", hc)][:, j, :],
                                        lt[:, j, cs],
                                        start=False,
                                        stop=(not first),
                                    )
                            else:
                                nc.tensor.matmul(
                                    pz[:],
                                    wsb[("xz", hc)][:, j, :],
                                    lt[:, j, :],
                                    start=(first and j == 0),
                                    stop=(not first and j == KOXZ - 1),
                                )

                    def run_epilogue(hc, pr, pz, pxn, phn, ht):
                        if last_pair and hc == c1:
                            # last chunk: epilogue in two 256-column pieces
                            # to shorten the post-matmul tail (4 tapered
                            # pieces measured 2.2us WORSE: per-op fixed
                            # costs exceed the smaller final transfer)
                            for lo in range(0, MBLK, 2 * P):
                                epilogue(b, hc, pr, pz, pxn, phn, ht, lo, lo + 2 * P)
                        else:
                            epilogue(b, hc, pr, pz, pxn, phn, ht, 0, MBLK)

                    ch0 = (c0, pr0, pz0, pxn0, phn0, ht0)
                    ch1 = (c1, pr1, pz1, pxn1, phn1, ht1)
                    for hc, pr, pz, pxn, phn, ht in (ch0, ch1):
                        hn_sweep(hc, phn)
                        hz_sweep(hc, pz, first=True)
                        r_sweep(hc, pr)
                    for hc, pr, pz, pxn, phn, ht in (ch0, ch1):
                        xn_sweep(hc, pxn)
                        xz_sweep(hc, pz, first=False, split_last=(hc == c1))
                        run_epilogue(hc, pr, pz, pxn, phn, ht)
                if lt_next is not None:
                    lt = lt_next
                    lt8 = lt8_next
                    lt_next = None
    nc.finalize()
    return nc


_PROGRAM_CACHE: dict = {}


def get_program(with_bias: bool) -> bass.Bass:
    if with_bias not in _PROGRAM_CACHE:
        _PROGRAM_CACHE[with_bias] = build_gru_program(with_bias)
    return _PROGRAM_CACHE[with_bias]


def prepare_in_maps(h, x, W_ir, W_iz, W_in, b_ir, b_iz, b_in, W_hr, W_hz, W_hn, b_hn):
    """Host-side shard + layout prep. Returns (in_maps, with_bias)."""
    h = np.ascontiguousarray(np.asarray(h, dtype=np.float32))
    x = np.ascontiguousarray(np.asarray(x, dtype=np.float32))
    assert x.shape == (B, F) and h.shape == (B, H), (x.shape, h.shape)

    import ml_dtypes

    fp8np = ml_dtypes.float8_e4m3
    # xz fp16 weights carry the same WS scale as the fp8 hz half so the z
    # PSUM accumulation is uniformly scaled (power-of-two: exact in fp16).
    # v12: the n-path (xn fp16 + hn fp8) is WS-scaled too — the tanh
    # activation scale rescales it for free and the fp8 hn part quantizes
    # in the e4m3 normal range.
    w_xz = (np.asarray(W_iz, np.float32) * WS).astype(np.float16)  # [F, H]
    w_xn = (np.asarray(W_in, np.float32) * WS).astype(np.float16)  # [F, H]
    wcat_r = np.concatenate([W_ir, W_hr], axis=0).astype(np.float32)  # [K, H]
    w_hn = np.asarray(W_hn, np.float32)  # [H, H]
    w_hz = np.asarray(W_hz, np.float32)  # [H, H]

    br = np.asarray(b_ir, np.float32)
    bz = np.asarray(b_iz, np.float32)
    bn = np.asarray(b_in, np.float32)
    bhn = np.asarray(b_hn, np.float32)
    biases = np.stack([br, bz, bn, bhn])  # [4, H]
    with_bias = bool(np.any(biases != 0.0))
    # b_hn is added to phn BEFORE the r-multiply, in the xWS-scaled domain
    biases[3] *= WS

    # per H-shard: weights in the exact SBUF layout
    wxz_shards = []
    wxn_shards = []
    w8_shards = []
    bias_shards = []
    for hj in range(H_SHARDS):
        cs = slice(hj * H_CORE, (hj + 1) * H_CORE)
        # [768, H_CORE] -> [KOXZ, P, HC_N, P] -> [HC_N, P, KOXZ, P]
        wxzs = w_xz[: KOXZ * P, cs].reshape(KOXZ, P, HC_N, P).transpose(2, 1, 0, 3)
        wxz_shards.append(np.ascontiguousarray(wxzs))
        wxns = w_xn[:, cs].reshape(KOX, P, HC_N, P).transpose(2, 1, 0, 3)
        wxn_shards.append(np.ascontiguousarray(wxns))
        # r + hn + hz + xz-quarter, fp8 DoubleRow layout [HC_N, P, W8_N, 2, P]
        w8 = np.empty((HC_N, P, W8_N, 2, P), fp8np)
        w8r_ = (wcat_r[:, cs] * WS).astype(fp8np)
        w8[:, :, :KO8] = w8r_.reshape(KO8, 2, P, HC_N, P).transpose(3, 2, 0, 1, 4)
        w8h_ = (w_hn[:, cs] * WS).astype(fp8np)
        w8[:, :, HN0:HZ0] = w8h_.reshape(KO8H, 2, P, HC_N, P).transpose(3, 2, 0, 1, 4)
        w8hz_ = (w_hz[:, cs] * WS).astype(fp8np)
        w8[:, :, HZ0:XZ8] = w8hz_.reshape(KO8H, 2, P, HC_N, P).transpose(
            3, 2, 0, 1, 4
        )
        # xz half: W_iz rows 512..1024, xWS (consistent with pz scale)
        w8xz_ = (np.asarray(W_iz, np.float32)[KOXZ * P :, cs] * WS).astype(fp8np)
        w8[:, :, XZ8:] = w8xz_.reshape(XZ8_N, 2, P, HC_N, P).transpose(
            3, 2, 0, 1, 4
        )
        w8_shards.append(np.ascontiguousarray(w8))
        if with_bias:
            # [4, H_CORE] -> [4, HC_N, P] -> [P, 4, HC_N]
            bs = biases[:, cs].reshape(4, HC_N, P).transpose(2, 0, 1)
            bias_shards.append(np.ascontiguousarray(bs.astype(np.float32)))

    # per batch-shard: lhsT blocks [NBLK, P, KOX, MBLK], fp8 copy, hT slices
    lhsT_shards = []
    lhsT8_shards = []
    hT_shards = []
    for bi in range(B_SHARDS):
        sl = slice(bi * B_CORE, (bi + 1) * B_CORE)
        # fp16 moving operand: x rows only (used by the xn / xz sweeps)
        # [F, B_CORE] -> [KOX, P, NBLK, MBLK] -> [NBLK, P, KOX, MBLK]
        ltx = x[sl].T.astype(np.float16)
        lt = ltx.reshape(KOX, P, NBLK, MBLK).transpose(2, 1, 0, 3)
        lhsT_shards.append(np.ascontiguousarray(lt))
        l8 = np.empty((K, B_CORE), fp8np)
        l8[:F] = x[sl].T.astype(fp8np)
        l8[F:] = h[sl].T.astype(fp8np)
        # [K, B_CORE] -> [KO8, 2, P, NBLK, MBLK] -> [NBLK, P, KO8, 2, MBLK]
        l8 = l8.reshape(KO8, 2, P, NBLK, MBLK).transpose(3, 2, 0, 1, 4)
        lhsT8_shards.append(np.ascontiguousarray(l8))
        hT_shards.append(np.ascontiguousarray(h[sl].T.astype(np.float16)))  # [H, B_CORE]

    in_maps = []
    for c in range(N_CORES):
        bi, hj = divmod(c, H_SHARDS)
        m = {
            "lhsT": lhsT_shards[bi],
            "lhsT8": lhsT8_shards[bi],
            "wxz": wxz_shards[hj],
            "wxn": wxn_shards[hj],
            "w8rh": w8_shards[hj],
            "hT": np.ascontiguousarray(
                hT_shards[bi][hj * H_CORE : (hj + 1) * H_CORE]
            ),
        }
        if with_bias:
            m["bias"] = bias_shards[hj]
        in_maps.append(m)
    return in_maps, with_bias


def kernel(h, x, W_ir, W_iz, W_in, b_ir, b_iz, b_in, W_hr, W_hz, W_hn, b_hn):
    in_maps, with_bias = prepare_in_maps(
        h, x, W_ir, W_iz, W_in, b_ir, b_iz, b_in, W_hr, W_hz, W_hn, b_hn
    )
    nc = get_program(with_bias)
    res = run_bass_kernel_spmd(nc, in_maps, list(range(N_CORES)))
    new_h = np.empty((B, H), np.float32)
    for c in range(N_CORES):
        bi, hj = divmod(c, H_SHARDS)
        outT = res.results[c]["out"]  # [H_CORE, B_CORE]
        new_h[bi * B_CORE : (bi + 1) * B_CORE, hj * H_CORE : (hj + 1) * H_CORE] = (
            outT.T
        )
    return (new_h, new_h)


# revision 67
# speedup vs baseline: 1.0039x; 1.0039x over previous
"""Trainium2 Bass kernel for a fused GRUCell step.

Math (reference):
    xi = x @ [W_ir W_iz W_in] + [b_ir b_iz b_in]
    hh = h @ [W_hr W_hz W_hn]
    r = sigmoid(xr + hr); z = sigmoid(xz + hz)
    n = tanh(xn + r * (hn + b_hn))
    new_h = (1 - z) * n + z * h

Strategy (v16, ~227.5us expected mean, rel err 1.98387e-2;
v11 242us,
v5 274.6us, v1 397us). HW exec noise between identical runs is
+-1.5-3us — judge changes on multi-run means, and only via the test.py
flow (one untraced run + one traced run per process; back-to-back
traced runs in one process measure ~45us slower for unknown
environmental reasons). Rel err is DETERMINISTIC (identical to 6
digits across runs) and the numpy sim (sim_precision.py method) now
predicts HW rel err to 6 digits at full batch — precision headroom can
be spent exactly to the 2e-2 gate.

- 2D shard — batch 4-way x hidden 2-way (8 cores), each core a
  [B_CORE=4096, H_CORE=512] output tile. Weights per core ~3.25MB.
  The GEMM runs with the *weights* stationary and 512-wide batch blocks
  moving. Outputs land transposed ([H-partition, batch-free]); h is
  supplied host-transposed and the final unshard transposes back.
  Per-gate biases become per-partition scalars, fused into ScalarE
  activations for free.
- Precision (v6): only the xz and xn sweeps stay fp16 (1 cycle/row on
  the PE); r (K=2048), hn (K=1024) and now hz (K=1024) run fp8(e4m3)
  with perf_mode=DoubleRow, packing K=256 per matmul — 2x PE throughput
  (HW-verified: DR matmuls issue at the same 216ns slot as fp16, so DR
  halves the slot count). 36 -> 32 slots per chunk. Weights are
  pre-scaled by WS=16 into the e4m3 normal range; the xz fp16 weights
  are also scaled by WS (exact, power of two) so the z PSUM
  accumulation is uniformly scaled and one sigmoid activation scale
  1/WS folds the rescale for free. Numpy-simulated rel err 1.763e-2
  against the 2e-2 gate (sim matched HW to ~4 digits at v5); full-fp8 z
  (2.18e-2) does not fit.
- The pair's LAST fp16 matmul is split 480+32 cols (v16): the short
  tail MM drains the PE pipe before the fp16->fp8 reconfigure, cutting
  each flip stall 619 -> 461ns (trace-verified, -2.4us total). The
  stall is drain-limited down to ~64-col tails; below that the
  residual ~245ns is LDWEIGHTS+reconfigure, irreducible.
- Chunks are processed in PAIRS sharing one DR block and one fp16
  block (v7): hn0 hz0 r0 hn1 hz1 r1 | xn0 xz0 xn1 xz1. The fp16->fp8
  PE mode flip costs ~500ns (HW-measured, lands on the 2nd matmul
  after the flip); pairing halves the flip count 32 -> 16. All eight
  PSUM banks are used single-buffered (pr/pz/pxn/phn x 2 chunks); the
  tile framework's slice-level tracking makes the next pair's sweeps
  wait only for the previous pair's epilogue reads of the same bank,
  which complete mid-pair (~0.7us margin, HW-verified no new stalls).
  z's PSUM accumulates across the hz (DR) and xz (fp16) sweeps into
  one bank and finishes last, so the whole tanh chain overlaps the xz
  sweep. hn leads the pair so the first real matmul of the kernel
  needs only 384KB of operands (w8 hn+hz part + lt8 h-half).
- Startup DMAs ride two HWDGE queues in parallel (v8): weights on the
  sync queue, lhsT blocks on the scalar (Activation) queue — the
  descriptor-issue instructions are ~650ns serialized per queue plus a
  ~2.6us ring latency, so splitting pulls the first real matmul's
  operand arrival ~3us earlier. (A second queue does NOT help steady
  bandwidth — one queue saturates the HBM port — and the scalar queue
  is idle during startup.)
- Weight/lhsT tiles beyond chunk 0 load as SINGLE whole-tile DMAs
  (4.6-8KB segments run the DMA engines ~1.5x faster than 2KB-segment
  halves); only chunk 0 keeps the split for first-matmul latency.
- DMA issue order is hand-scheduled on the sync queue (one FIFO,
  ~380GB/s measured once ramped, bulk flow from ~8us): w8(0) is split
  in two so the hn/hz part lands first, then the lt8 h-half, the r
  part, lt8 x-half, wxn, lt halves, wxz. Remaining weights stream in
  consumption order during block 0, lhsT blocks prefetch mid-block.
- ~48 warmup matmuls on a zeroed scratch tile run during the startup
  DMA wait so the PE HAM clock gate is at 2.4GHz (not 1.2) when the
  real stream starts. v5 used 100 and the warmup tail (15.3us) delayed
  the real stream past operand arrival (~10.8us). All four PSUM pool
  tags must be rotated together: rotating only one puts it anti-phase
  with the others and serializes every chunk against the epilogue
  (+50us when tried).
- Blocks walk the hidden chunks in snake order (odd blocks reversed),
  so block 1's first chunk needs the weights that arrive last — this
  removed a 4.4us stall at the block 0->1 boundary.
- The last chunk's epilogue runs in two 256-column pieces to shorten
  the post-matmul tail.
- Epilogue (v11): sr/sz/sn/tt intermediates are fp16 (2x DVE
  throughput, |r|,|z|,|n|<=1 so rounding is ~3e-4). Six ops + three
  activations per chunk.
- v12/v13: the ENTIRE n-path (hn fp8 + xn fp16 weights) carries the
  WS scale and one tanh activation scale folds the 1/WS for free, so
  hn quantizes in the e4m3 normal range with no rescale op. The last
  512 K-rows of the xz GEMM moved into the DR block as 2 extra fp8
  chunks per hidden chunk (the fp8 moving slices are already in SBUF
  for the r sweep): 32 -> 30 slots per chunk, -13.8us. This spends
  the precision budget to 1.98387e-2 vs the 2e-2 gate — deterministic,
  sim-verified to 6 digits at full batch; a third xz quarter (2.08e-2)
  and an xn quarter (2.04e-2) do NOT fit.

Accumulation is fp32 in PSUM throughout: pr/pz/pxn/phn x 2 chunks of
the active pair = all 8 banks, single-buffered.

Measured anatomy at v11 (exec ~242us): entry barrier + engine preamble
to ~7.2us, first real matmul ~12.8us (DMA-bandwidth-bound startup:
pair 0 consumes 4.25MB at the ~0.31MB/us early DMA rate), matmul
stream 1024 x 216ns + ~6us of flip/boundary stalls, ends ~235us;
epilogue+out-DMA tail ~3.5us; fixed teardown barrier ~4us.
"""

import os
import sys

import numpy as np

sys.path.insert(0, "/opt/trn_rl_repo")
os.environ.setdefault("MYCRO_LOCAL_CACHE", "1")

import concourse.bass as bass  # noqa: E402
import concourse.mybir as mybir  # noqa: E402
import concourse.tile as tile  # noqa: E402
from concourse import bacc  # noqa: E402
from concourse.bass_utils import run_bass_kernel_spmd  # noqa: E402

N_CORES = 8
B_SHARDS = 4
H_SHARDS = 2
B = 16384
F = 1024  # input feature dim
H = 1024  # hidden dim
K = F + H  # full GEMM contraction dim (x features then h features)
P = 128
KOX = F // P  # 8 k-chunks of 128 for the fp16 (x-side) sweeps
B_CORE = B // B_SHARDS  # 4096
H_CORE = H // H_SHARDS  # 512
MBLK = 512  # batch columns per moving-operand block (= PSUM bank width)
NBLK = B_CORE // MBLK  # 8
HC_N = H_CORE // P  # 4 hidden-column chunks of 128 (PSUM partition dim)
KO8 = K // 256  # 8 k-chunks of 256 for fp8 DoubleRow (r gate, full K)
KO8X = F // 256  # 4 fp8 k-chunks belonging to the x part
KO8H = H // 256  # 4 fp8 k-chunks for the hn / hz parts
XZ8_N = 2  # fp8 DR chunks of the xz part (x rows 512..1024, v13)
W8_N = KO8 + 2 * KO8H + XZ8_N  # 18 fp8 weight chunks: r | hn | hz | xz-half
HN0 = KO8  # first hn chunk index in w8rh
HZ0 = KO8 + KO8H  # first hz chunk index in w8rh
XZ8 = HZ0 + KO8H  # first xz fp8 chunk index in w8rh
KOXZ = KOX - 2 * XZ8_N  # 4 fp16 xz k-chunks (rows 0..512; the rest fp8 DR)
WS = 16.0  # fp8 weight scale: lifts w std ~0.031 into e4m3 normal range


def build_gru_program(with_bias: bool) -> bass.Bass:
    """One SPMD program; every core runs it on its own (batch, hidden) tile."""
    fp16 = mybir.dt.float16
    f32 = mybir.dt.float32

    # Bacc (not plain Bass): its compile pipeline splits multi-sem waits into
    # event semaphores — walrus rejects >1 wait on most engine instructions.
    nc = bacc.Bacc()
    # Host-prearranged so every DMA lands contiguously per partition:
    # lhsT[b, p, ko, m] = x.T[ko*P+p, b*MBLK+m]  (x rows only; fp16 sweeps)
    lhsT = nc.declare_dram_parameter("lhsT", [NBLK, P, KOX, MBLK], fp16, isOutput=False)
    fp8 = mybir.dt.float8e4
    # xz and xn (first F rows of z / n gates) stay fp16; xz pre-scaled by WS
    wxz = nc.declare_dram_parameter("wxz", [HC_N, P, KOXZ, P], fp16, isOutput=False)
    wxn = nc.declare_dram_parameter("wxn", [HC_N, P, KOX, P], fp16, isOutput=False)
    # r gate (full K) + hn + hz parts in fp8 DoubleRow layout, combined:
    # j in [0,KO8):      WS*Wcat_r[j*256+i*128+p, hc*P+n]
    # j in [HN0,HN0+4):  W_hn[(j-HN0)*256+i*128+p, hc*P+n]   (unscaled)
    # j in [HZ0,HZ0+4):  WS*W_hz[(j-HZ0)*256+i*128+p, hc*P+n]
    w8rh = nc.declare_dram_parameter("w8rh", [HC_N, P, W8_N, 2, P], fp8, isOutput=False)
    # fp8 copy of the moving operand (full K), DoubleRow-paired
    lhsT8 = nc.declare_dram_parameter(
        "lhsT8", [NBLK, P, KO8, 2, MBLK], fp8, isOutput=False
    )
    # hT[j, m] = h_shard[m, j]  (host-transposed h slice; fp16 — only feeds
    # the z*h blend, where fp16 rounding of h is ~3e-4 relative)
    hT = nc.declare_dram_parameter("hT", [H_CORE, B_CORE], fp16, isOutput=False)
    if with_bias:
        # bias[p, g, hc] = b_g[hj*H_CORE + hc*P + p]; g: 0=b_ir 1=b_iz 2=b_in 3=b_hn
        biasp = nc.declare_dram_parameter("bias", [P, 4, HC_N], f32, isOutput=False)
    out = nc.declare_dram_parameter("out", [H_CORE, B_CORE], f32, isOutput=True)

    Sigmoid = mybir.ActivationFunctionType.Sigmoid
    Tanh = mybir.ActivationFunctionType.Tanh
    DR = mybir.MatmulPerfMode.DoubleRow

    with tile.TileContext(nc) as tc:
        with (
            tc.tile_pool(name="wpool", bufs=1) as wpool,
            tc.tile_pool(name="lpool", bufs=3) as lpool,
            tc.tile_pool(name="hpool", bufs=2) as hpool,
            tc.tile_pool(name="opool", bufs=3) as opool,
            tc.tile_pool(name="epool", bufs=2) as epool,
            tc.tile_pool(name="psum", bufs=1, space="PSUM") as psum,
        ):
            wsb = {}
            w8sb = {}

            def load_wxz(hc: int):
                t = wpool.tile([P, KOXZ, P], fp16, tag=f"wxz_{hc}")
                nc.sync.dma_start(t[:], wxz[hc])
                wsb[("xz", hc)] = t

            def load_wxn(hc: int):
                t = wpool.tile([P, KOX, P], fp16, tag=f"wxn_{hc}")
                nc.sync.dma_start(t[:], wxn[hc])
                wsb[("xn", hc)] = t

            def load_w8(hc: int, part: int, engine: str = "sync"):
                # part 1 = hn+hz+xz8 chunks (consumed first), part 0 = r
                # chunks, part 2 = whole tile in ONE dma (4.6KB segments run
                # the DMA engines ~1.5x faster than the 2KB-segment halves —
                # use for every chunk except the startup-critical chunk 0)
                if hc not in w8sb:
                    t = wpool.tile([P, W8_N, 2, P], fp8, tag=f"w8_{hc}")
                    w8sb[hc] = t
                t = w8sb[hc]
                eng = getattr(nc, engine)
                if part == 2:
                    eng.dma_start(t[:], w8rh[hc])
                elif part == 1:
                    eng.dma_start(t[:, HN0:W8_N, :, :], w8rh[hc, :, HN0:W8_N, :, :])
                else:
                    eng.dma_start(t[:, 0:HN0, :, :], w8rh[hc, :, 0:HN0, :, :])

            def load_lt(b: int):
                # two ko-halves so the xn sweep can start after 0.5MB
                t = lpool.tile([P, KOX, MBLK], fp16, tag="lt")
                half = KOX // 2
                nc.sync.dma_start(t[:, 0:half, :], lhsT[b, :, 0:half, :])
                nc.sync.dma_start(t[:, half:KOX, :], lhsT[b, :, half:KOX, :])
                return t

            # --- startup-critical DMA order ---
            # Two HWDGE queues issue in parallel (descriptor-issue is ~650ns
            # serialized per queue + ~2.6us ring latency): the sync queue
            # carries the weights, the scalar (Activation) queue carries the
            # lhsT blocks. Pair 0 runs hn(0) first: needs w8(0) part1 +
            # lt8 h-half = 384KB, one descriptor each on the two queues.
            # chunk 1's weights follow since the pair consumes them ~3.5us in.
            lt8 = lpool.tile([P, KO8, 2, MBLK], fp8, tag="lt8")
            # (GpSimd SWDGE for these two loads was tried — its engine exits
            # the preamble earlier but software descriptor generation is far
            # slower: 250.7us vs 243.9us. Stay on the HWDGE queues.)
            # (Splitting these first loads into smaller first-consumed slices
            # moved the first real matmul 11.5us <- 12.8us but the startup
            # window is DMA-BANDWIDTH-bound — pair 0 consumes 4.25MB at the
            # ~0.31MB/us early rate — so the extra descriptors only delayed
            # the later transfers: 245.3us vs 242.3us mean. Keep it coarse.)
            load_w8(0, 1)
            nc.scalar.dma_start(lt8[:, KO8X:KO8, :, :], lhsT8[0, :, KO8X:KO8, :, :])
            load_w8(0, 0)
            nc.scalar.dma_start(lt8[:, 0:KO8X, :, :], lhsT8[0, :, 0:KO8X, :, :])
            load_w8(1, 2)
            lt = lpool.tile([P, KOX, MBLK], fp16, tag="lt")
            nc.scalar.dma_start(lt[:, 0 : KOX // 2, :], lhsT[0, :, 0 : KOX // 2, :])
            nc.scalar.dma_start(lt[:, KOX // 2 : KOX, :], lhsT[0, :, KOX // 2 : KOX, :])
            load_wxn(0)
            load_wxz(0)
            load_wxn(1)
            load_wxz(1)
            bias_sb = None
            if with_bias:
                bias_sb = wpool.tile([P, 4, HC_N], f32, tag="bias_sb")
                nc.sync.dma_start(bias_sb[:], biasp[:])

            ht_tiles = {}

            def load_ht(b: int, hc: int):
                t = hpool.tile([P, MBLK], fp16, tag=f"ht{hc}")
                nc.sync.dma_start(
                    t[:], hT[hc * P : (hc + 1) * P, b * MBLK : (b + 1) * MBLK]
                )
                ht_tiles[(b, hc)] = t

            load_ht(0, 0)
            load_ht(0, 1)

            # PE warmup: ~48 tiny matmuls on a zeroed scratch tile while the
            # first operands stream in. Keeps the PE HAM activity window busy
            # so it unthrottles from 1.2GHz to 2.4GHz before the real matmul
            # stream starts, and ends (~11.6us) just as the operands land.
            # fp8 DR warmup (v15): matches the mode of the stream-opening DR
            # sweep. (The ~430ns slot-1 stall survives regardless — it is a
            # stream-start pipeline effect, NOT the fp16->fp8 flip; timing
            # measured identical to the fp16 warmup, kept for mode hygiene.)
            warm_w = wpool.tile([P, 2, P], fp8, tag="warm_w")
            nc.vector.memset(warm_w[:], 0.0)
            warm_ps = psum.tile([P, MBLK], f32, tag="pr0")
            for _ in range(48):
                nc.tensor.matmul(
                    warm_ps[:, 0:P],
                    warm_w[:],
                    warm_w[:, :, 0:P],
                    start=True,
                    stop=True,
                    perf_mode=DR,
                )

            def epilogue(b, hc, pr, pz, pxn, phn, ht, lo, hi):
                s = slice(lo, hi)
                # fp16 intermediates: 2x DVE throughput on the gate chain;
                # |r|,|z|,|n| <= 1 so fp16 rounding adds ~3e-4 rel (sim:
                # 1.7634e-2 vs 1.7630e-2). hn weights are unscaled (WS=1,
                # sim 1.790e-2) so the old 1/WS rescale op disappears.
                sr = epool.tile([P, MBLK], fp16, tag="sr")
                sz = epool.tile([P, MBLK], fp16, tag="sz")
                sn = epool.tile([P, MBLK], fp16, tag="sn")
                tt = epool.tile([P, MBLK], fp16, tag="tt")
                ot = opool.tile([P, MBLK], f32, tag="ot")
                # issue order matters: ScalarE/DVE are strict FIFO, so sz
                # (which waits on the last-finishing xz-sweep) is issued
                # after tanh to avoid blocking the chain
                if with_bias:
                    nc.scalar.activation(
                        sr[:, s],
                        pr[:, s],
                        Sigmoid,
                        bias=bias_sb[:, 0, hc : hc + 1],
                        scale=1.0 / WS,
                    )
                    nc.vector.tensor_scalar(
                        tt[:, s],
                        phn[:, s],
                        1.0,
                        bias_sb[:, 3, hc : hc + 1],
                        mybir.AluOpType.mult,
                        mybir.AluOpType.add,
                    )
                    nc.vector.tensor_mul(tt[:, s], sr[:, s], tt[:, s])
                    nc.vector.tensor_add(tt[:, s], tt[:, s], pxn[:, s])
                    nc.scalar.activation(
                        sn[:, s],
                        tt[:, s],
                        Tanh,
                        bias=bias_sb[:, 2, hc : hc + 1],
                        scale=1.0 / WS,
                    )
                    nc.vector.tensor_sub(tt[:, s], ht[:, s], sn[:, s])
                    nc.scalar.activation(
                        sz[:, s],
                        pz[:, s],
                        Sigmoid,
                        bias=bias_sb[:, 1, hc : hc + 1],
                        scale=1.0 / WS,
                    )
                else:
                    nc.scalar.activation(sr[:, s], pr[:, s], Sigmoid, scale=1.0 / WS)
                    nc.vector.tensor_mul(tt[:, s], sr[:, s], phn[:, s])
                    nc.vector.tensor_add(tt[:, s], tt[:, s], pxn[:, s])
                    # the whole n-path (hn, xn) carries the WS weight scale;
                    # one tanh activation scale rescales it for free (v12)
                    nc.scalar.activation(sn[:, s], tt[:, s], Tanh, scale=1.0 / WS)
                    nc.vector.tensor_sub(tt[:, s], ht[:, s], sn[:, s])
                    nc.scalar.activation(sz[:, s], pz[:, s], Sigmoid, scale=1.0 / WS)
                nc.vector.tensor_mul(tt[:, s], tt[:, s], sz[:, s])
                nc.vector.tensor_add(ot[:, s], sn[:, s], tt[:, s])
                nc.sync.dma_start(
                    out[hc * P : (hc + 1) * P, b * MBLK + lo : b * MBLK + hi],
                    ot[:, s],
                )

            def get_ht(b, hc):
                if (b, hc) in ht_tiles:
                    return ht_tiles.pop((b, hc))
                t = hpool.tile([P, MBLK], fp16, tag=f"ht{hc}")
                nc.sync.dma_start(
                    t[:], hT[hc * P : (hc + 1) * P, b * MBLK : (b + 1) * MBLK]
                )
                return t

            lt_next = None
            lt8_next = None
            for b in range(NBLK):
                # snake order: odd blocks walk the pairs in reverse, so the
                # first pair of block 1 needs the weights that arrive last
                prs = [(0, 1), (2, 3)] if b % 2 == 0 else [(3, 2), (1, 0)]
                for pi, (c0, c1) in enumerate(prs):
                    if b == 0 and pi == 0:
                        # pull in the remaining weights + early h tiles while
                        # pair 0 runs
                        for hc in (2, 3):
                            load_w8(hc, 2)
                            load_wxn(hc)
                            load_wxz(hc)
                        load_ht(0, 2)
                        load_ht(0, 3)
                        load_ht(1, 3)  # block 1 starts at hc=3 (snake)
                        load_ht(1, 2)
                    # prefetch next batch block during the second pair
                    if pi == 1 and b + 1 < NBLK:
                        # single whole-tile DMAs: 8KB segments, ~1.5x faster
                        # per DMA engine than the 2-4KB split halves (the
                        # halves only matter for the startup-critical block 0)
                        lt8_next = lpool.tile([P, KO8, 2, MBLK], fp8, tag="lt8")
                        nc.sync.dma_start(lt8_next[:], lhsT8[b + 1])
                        lt_next = lpool.tile([P, KOX, MBLK], fp16, tag="lt")
                        nc.sync.dma_start(lt_next[:], lhsT[b + 1])

                    ht0 = get_ht(b, c0)
                    ht1 = get_ht(b, c1)

                    pr0 = psum.tile([P, MBLK], f32, tag="pr0")
                    pz0 = psum.tile([P, MBLK], f32, tag="pz0")
                    pxn0 = psum.tile([P, MBLK], f32, tag="pxn0")
                    phn0 = psum.tile([P, MBLK], f32, tag="phn0")
                    pr1 = psum.tile([P, MBLK], f32, tag="pr1")
                    pz1 = psum.tile([P, MBLK], f32, tag="pz1")
                    pxn1 = psum.tile([P, MBLK], f32, tag="pxn1")
                    phn1 = psum.tile([P, MBLK], f32, tag="phn1")

                    # (v9/v10 tried alternating DR-first / fp16-first pairs
                    # to halve the ~620ns fp16->fp8 PE flips; the fp16-first
                    # pairs defer their epilogues past the pair end and the
                    # resulting back-pressure cost more than the flips saved:
                    # 245.2us vs 242.3us. Keep every pair DR-first.)
                    last_pair = b == NBLK - 1 and pi == len(prs) - 1

                    def hn_sweep(hc, phn):
                        for j in range(KO8H):
                            nc.tensor.matmul(
                                phn[:],
                                w8sb[hc][:, HN0 + j, :, :],
                                lt8[:, KO8X + j, :, :],
                                start=(j == 0),
                                stop=(j == KO8H - 1),
                                perf_mode=DR,
                            )

                    def hz_sweep(hc, pz, first):
                        for j in range(KO8H):
                            nc.tensor.matmul(
                                pz[:],
                                w8sb[hc][:, HZ0 + j, :, :],
                                lt8[:, KO8X + j, :, :],
                                start=(first and j == 0),
                                stop=False,
                                perf_mode=DR,
                            )
                        # v12/v13: the last 512 K-rows of the xz part also
                        # run fp8 DR here (weights xWS like everything in
                        # pz), trimming the fp16 xz sweep 8 -> 4 matmuls.
                        # The moving slices (x rows 512..1024) are already
                        # in SBUF for the r sweep.
                        for j2 in range(XZ8_N):
                            nc.tensor.matmul(
                                pz[:],
                                w8sb[hc][:, XZ8 + j2, :, :],
                                lt8[:, KO8X - XZ8_N + j2, :, :],
                                start=False,
                                stop=False,
                                perf_mode=DR,
                            )

                    def r_sweep(hc, pr):
                        for j in range(KO8):
                            nc.tensor.matmul(
                                pr[:],
                                w8sb[hc][:, j, :, :],
                                lt8[:, j, :, :],
                                start=(j == 0),
                                stop=(j == KO8 - 1),
                                perf_mode=DR,
                            )

                    def xn_sweep(hc, pxn):
                        for j in range(KOX):
                            nc.tensor.matmul(
                                pxn[:],
                                wsb[("xn", hc)][:, j, :],
                                lt[:, j, :],
                                start=(j == 0),
                                stop=(j == KOX - 1),
                            )

                    def xz_sweep(hc, pz, first, split_last=False):
                        for j in range(KOXZ):
                            if split_last and j == KOXZ - 1:
                                # the pair's LAST fp16 matmul runs 448+64
                                # cols: the short tail MM drains the PE pipe
                                # ~190ns sooner, shrinking the fp16->fp8
                                # reconfigure stall of the next pair's DR
                                # block (if the stall is drain-limited)
                                for cs in (slice(0, 480), slice(480, 512)):
                                    nc.tensor.matmul(
                                        pz[:, cs],
                                        wsb[("xz", hc)][:, j, :],
                                        lt[:, j, cs],
                                        start=False,
                                        stop=(not first),
                                    )
                            else:
                                nc.tensor.matmul(
                                    pz[:],
                                    wsb[("xz", hc)][:, j, :],
                                    lt[:, j, :],
                                    start=(first and j == 0),
                                    stop=(not first and j == KOXZ - 1),
                                )

                    def run_epilogue(hc, pr, pz, pxn, phn, ht):
                        if last_pair and hc == c1:
                            # last chunk: epilogue in two 256-column pieces
                            # to shorten the post-matmul tail (4 tapered
                            # pieces measured 2.2us WORSE: per-op fixed
                            # costs exceed the smaller final transfer)
                            for lo in range(0, MBLK, 2 * P):
                                epilogue(b, hc, pr, pz, pxn, phn, ht, lo, lo + 2 * P)
                        else:
                            epilogue(b, hc, pr, pz, pxn, phn, ht, 0, MBLK)

                    ch0 = (c0, pr0, pz0, pxn0, phn0, ht0)
                    ch1 = (c1, pr1, pz1, pxn1, phn1, ht1)
                    for hc, pr, pz, pxn, phn, ht in (ch0, ch1):
                        hn_sweep(hc, phn)
                        hz_sweep(hc, pz, first=True)
                        r_sweep(hc, pr)
                    for hc, pr, pz, pxn, phn, ht in (ch0, ch1):
                        xn_sweep(hc, pxn)
                        xz_sweep(hc, pz, first=False, split_last=(hc == c1))
                        run_epilogue(hc, pr, pz, pxn, phn, ht)
                if lt_next is not None:
                    lt = lt_next
                    lt8 = lt8_next
                    lt_next = None
    nc.finalize()
    return nc


_PROGRAM_CACHE: dict = {}


def get_program(with_bias: bool) -> bass.Bass:
    if with_bias not in _PROGRAM_CACHE:
        _PROGRAM_CACHE[with_bias] = build_gru_program(with_bias)
    return _PROGRAM_CACHE[with_bias]


def prepare_in_maps(h, x, W_ir, W_iz, W_in, b_ir, b_iz, b_in, W_hr, W_hz, W_hn, b_hn):
    """Host-side shard + layout prep. Returns (in_maps, with_bias)."""
    h = np.ascontiguousarray(np.asarray(h, dtype=np.float32))
    x = np.ascontiguousarray(np.asarray(x, dtype=np.float32))
    assert x.shape == (B, F) and h.shape == (B, H), (x.shape, h.shape)

    import ml_dtypes

    fp8np = ml_dtypes.float8_e4m3
    # xz fp16 weights carry the same WS scale as the fp8 hz half so the z
    # PSUM accumulation is uniformly scaled (power-of-two: exact in fp16).
    # v12: the n-path (xn fp16 + hn fp8) is WS-scaled too — the tanh
    # activation scale rescales it for free and the fp8 hn part quantizes
    # in the e4m3 normal range.
    w_xz = (np.asarray(W_iz, np.float32) * WS).astype(np.float16)  # [F, H]
    w_xn = (np.asarray(W_in, np.float32) * WS).astype(np.float16)  # [F, H]
    wcat_r = np.concatenate([W_ir, W_hr], axis=0).astype(np.float32)  # [K, H]
    w_hn = np.asarray(W_hn, np.float32)  # [H, H]
    w_hz = np.asarray(W_hz, np.float32)  # [H, H]

    br = np.asarray(b_ir, np.float32)
    bz = np.asarray(b_iz, np.float32)
    bn = np.asarray(b_in, np.float32)
    bhn = np.asarray(b_hn, np.float32)
    biases = np.stack([br, bz, bn, bhn])  # [4, H]
    with_bias = bool(np.any(biases != 0.0))
    # b_hn is added to phn BEFORE the r-multiply, in the xWS-scaled domain
    biases[3] *= WS

    # per H-shard: weights in the exact SBUF layout
    wxz_shards = []
    wxn_shards = []
    w8_shards = []
    bias_shards = []
    for hj in range(H_SHARDS):
        cs = slice(hj * H_CORE, (hj + 1) * H_CORE)
        # [768, H_CORE] -> [KOXZ, P, HC_N, P] -> [HC_N, P, KOXZ, P]
        wxzs = w_xz[: KOXZ * P, cs].reshape(KOXZ, P, HC_N, P).transpose(2, 1, 0, 3)
        wxz_shards.append(np.ascontiguousarray(wxzs))
        wxns = w_xn[:, cs].reshape(KOX, P, HC_N, P).transpose(2, 1, 0, 3)
        wxn_shards.append(np.ascontiguousarray(wxns))
        # r + hn + hz + xz-quarter, fp8 DoubleRow layout [HC_N, P, W8_N, 2, P]
        w8 = np.empty((HC_N, P, W8_N, 2, P), fp8np)
        w8r_ = (wcat_r[:, cs] * WS).astype(fp8np)
        w8[:, :, :KO8] = w8r_.reshape(KO8, 2, P, HC_N, P).transpose(3, 2, 0, 1, 4)
        w8h_ = (w_hn[:, cs] * WS).astype(fp8np)
        w8[:, :, HN0:HZ0] = w8h_.reshape(KO8H, 2, P, HC_N, P).transpose(3, 2, 0, 1, 4)
        w8hz_ = (w_hz[:, cs] * WS).astype(fp8np)
        w8[:, :, HZ0:XZ8] = w8hz_.reshape(KO8H, 2, P, HC_N, P).transpose(
            3, 2, 0, 1, 4
        )
        # xz half: W_iz rows 512..1024, xWS (consistent with pz scale)
        w8xz_ = (np.asarray(W_iz, np.float32)[KOXZ * P :, cs] * WS).astype(fp8np)
        w8[:, :, XZ8:] = w8xz_.reshape(XZ8_N, 2, P, HC_N, P).transpose(
            3, 2, 0, 1, 4
        )
        w8_shards.append(np.ascontiguousarray(w8))
        if with_bias:
            # [4, H_CORE] -> [4, HC_N, P] -> [P, 4, HC_N]
            bs = biases[:, cs].reshape(4, HC_N, P).transpose(2, 0, 1)
            bias_shards.append(np.ascontiguousarray(bs.astype(np.float32)))

    # per batch-shard: lhsT blocks [NBLK, P, KOX, MBLK], fp8 copy, hT slices
    lhsT_shards = []
    lhsT8_shards = []
    hT_shards = []
    for bi in range(B_SHARDS):
        sl = slice(bi * B_CORE, (bi + 1) * B_CORE)
        # fp16 moving operand: x rows only (used by the xn / xz sweeps)
        # [F, B_CORE] -> [KOX, P, NBLK, MBLK] -> [NBLK, P, KOX, MBLK]
        ltx = x[sl].T.astype(np.float16)
        lt = ltx.reshape(KOX, P, NBLK, MBLK).transpose(2, 1, 0, 3)
        lhsT_shards.append(np.ascontiguousarray(lt))
        l8 = np.empty((K, B_CORE), fp8np)
        l8[:F] = x[sl].T.astype(fp8np)
        l8[F:] = h[sl].T.astype(fp8np)
        # [K, B_CORE] -> [KO8, 2, P, NBLK, MBLK] -> [NBLK, P, KO8, 2, MBLK]
        l8 = l8.reshape(KO8, 2, P, NBLK, MBLK).transpose(3, 2, 0, 1, 4)
        lhsT8_shards.append(np.ascontiguousarray(l8))
        hT_shards.append(np.ascontiguousarray(h[sl].T.astype(np.float16)))  # [H, B_CORE]

    in_maps = []
    for c in range(N_CORES):
        bi, hj = divmod(c, H_SHARDS)
        m = {
            "lhsT": lhsT_shards[bi],
            "lhsT8": lhsT8_shards[bi],
            "wxz": wxz_shards[hj],
            "wxn": wxn_shards[hj],
            "w8rh": w8_shards[hj],
            "hT": np.ascontiguousarray(
                hT_shards[bi][hj * H_CORE : (hj + 1) * H_CORE]
            ),
        }
        if with_bias:
            m["bias"] = bias_shards[hj]
        in_maps.append(m)
    return in_maps, with_bias


def kernel(h, x, W_ir, W_iz, W_in, b_ir, b_iz, b_in, W_hr, W_hz, W_hn, b_hn):
    in_maps, with_bias = prepare_in_maps(
        h, x, W_ir, W_iz, W_in, b_ir, b_iz, b_in, W_hr, W_hz, W_hn, b_hn
    )
    nc = get_program(with_bias)
    res = run_bass_kernel_spmd(nc, in_maps, list(range(N_CORES)))
    new_h = np.empty((B, H), np.float32)
    for c in range(N_CORES):
        bi, hj = divmod(c, H_SHARDS)
        outT = res.results[c]["out"]  # [H_CORE, B_CORE]
        new_h[bi * B_CORE : (bi + 1) * B_CORE, hj * H_CORE : (hj + 1) * H_CORE] = (
            outT.T
        )
    return (new_h, new_h)
